# revision 1
# baseline (speedup 1.0000x reference)
"""LocalRNN Trainium2 kernel: GLU -> pointwise conv -> 9-step windowed LSTM.

Full inputs in, full output out. Sharding: batch across 8 cores (2 batches/core).

v2 design notes:
- All matmuls float32r (sustains ~1 cycle/row at N=512).
- Everything on-chip in transposed [feature, token] layout; the recurrence
  needs no transposes (h is produced by DVE directly in matmul-rhs layout).
- Conv (kernel_size=1) folded into W_ih on the host: G = (W_ih@conv_w) @ u
  with bias b_ih+b_hh+W_ih@conv_b.
- Gate rows permuted host-side to I,F,O,G so one sigmoid covers 1536
  contiguous PSUM columns.
- Input-side gates G computed once per token (9x reuse across overlapping
  windows); per-step G slice enters PSUM via an identity matmul in the same
  accumulation group as the W_hh matmuls.
- Output returned in transposed layout; host does the final transpose.
"""
from contextlib import ExitStack

import numpy as np

import concourse.bass as bass
import concourse.mybir as mybir
import concourse.tile as tile
from concourse import bacc, bass_utils
from concourse.masks import make_identity

F32 = mybir.dt.float32
F32R = mybir.dt.float32r
AF = mybir.ActivationFunctionType

N_CORES = 8
B_PER_CORE = 2          # batches per core
L = 512                 # sequence length
NT = B_PER_CORE * L     # tokens per core = 1024
D = 512                 # model dim
DH = 256                # GLU half dim
G4 = 4 * D              # 2048 gate rows
K = 9                   # window size
PAD = K - 1             # 8
LW = PAD + L            # 520: per-batch padded G row width

_cache = {}


def _build():
    nc = bacc.Bacc(
        trn_type="TRN2", target_bir_lowering=False, debug=False, num_devices=N_CORES
    )

    x_d = nc.dram_tensor("x", [NT, D], F32, kind="ExternalInput").ap()
    wf_d = nc.dram_tensor("wf", [DH, G4], F32, kind="ExternalInput").ap()    # (w_ih@conv_w).T permuted
    whh_d = nc.dram_tensor("whh", [D, G4], F32, kind="ExternalInput").ap()   # w_hh.T permuted
    bias_d = nc.dram_tensor("bias", [128, 32], F32, kind="ExternalInput").ap()
    out_d = nc.dram_tensor("out", [D, NT], F32, kind="ExternalOutput").ap()  # transposed out

    with tile.TileContext(nc) as tc, ExitStack() as top:
        const_pool = top.enter_context(tc.tile_pool(name="const", bufs=1))
        w_pool = top.enter_context(tc.tile_pool(name="weights", bufs=1))
        state_pool = top.enter_context(tc.tile_pool(name="state", bufs=1))

        ident_f32 = const_pool.tile([128, 128], F32, tag="idf")
        make_identity(nc, ident_f32[:])
        ident = const_pool.tile([128, 128], F32R, tag="idr")
        nc.scalar.copy(ident[:], ident_f32[:])
        zeros8 = const_pool.tile([128, PAD], F32, tag="z8")
        nc.gpsimd.memset(zeros8[:], 0.0)
        bias_sb = const_pool.tile([128, 32], F32, tag="bias")
        nc.sync.dma_start(bias_sb[:], bias_d)

        whh = [w_pool.tile([128, G4], F32R, tag=f"whh{dk}", name=f"whh{dk}")
               for dk in range(4)]

        gt = [state_pool.tile([128, B_PER_CORE * LW], F32R, tag=f"gt{i}", name=f"gt{i}")
              for i in range(16)]
        hT = [[state_pool.tile([128, NT], F32R, tag=f"h{p}_{j}", name=f"h{p}_{j}")
               for j in range(4)] for p in range(2)]
        cT = [state_pool.tile([128, NT], F32, tag=f"c{j}", name=f"c{j}") for j in range(4)]

        tp = top.enter_context(tc.tile_pool(name="tmp", bufs=2))

        def cell0(j, b):
            """step 0: c = sig(I)*tanh(G); h = sig(O)*tanh(c)."""
            cs = cT[j][:, b * 512:(b + 1) * 512]
            hs = hT[0][j][:, b * 512:(b + 1) * 512]
            g0 = b * LW  # step-0 slice offset (pad col 0..7 + G cols 0..503)
            tI = tp.tile([128, 512], F32, tag="t1", name="tI0")
            nc.scalar.activation(tI[:], gt[0 * 4 + j][:, g0:g0 + 512], AF.Sigmoid)
            tG = tp.tile([128, 512], F32, tag="tG", name="tG0")
            nc.scalar.activation(tG[:], gt[3 * 4 + j][:, g0:g0 + 512], AF.Tanh)
            tO = tp.tile([128, 512], F32, tag="tSig", name="tO0")
            nc.scalar.activation(tO[:], gt[2 * 4 + j][:, g0:g0 + 512], AF.Sigmoid)
            nc.vector.tensor_mul(cs, tI[:], tG[:])
            tTc = tp.tile([128, 512], F32, tag="tTc", name="tTc0")
            nc.scalar.activation(tTc[:], cs, AF.Tanh)
            nc.vector.tensor_mul(hs, tO[:], tTc[:])

        def cell(j, b, P, k):
            """steps 1..8: full LSTM cell from psum P [128, 2048] = I|F|O|G."""
            cs = cT[j][:, b * 512:(b + 1) * 512]
            hs = hT[k % 2][j][:, b * 512:(b + 1) * 512]
            tSig = tp.tile([128, 1536], F32, tag="tSig", name="tSig")
            nc.scalar.activation(tSig[:], P[:, 0:1536], AF.Sigmoid)
            tG = tp.tile([128, 512], F32, tag="tG", name="tG")
            nc.scalar.activation(tG[:], P[:, 1536:2048], AF.Tanh)
            t1 = tp.tile([128, 512], F32, tag="t1", name="t1")
            nc.vector.tensor_mul(t1[:], tSig[:, 0:512], tG[:])
            t2 = tp.tile([128, 512], F32, tag="tG", name="t2")
            nc.vector.tensor_mul(t2[:], tSig[:, 512:1024], cs)
            nc.vector.tensor_add(cs, t1[:], t2[:])
            tTc = tp.tile([128, 512], F32, tag="tTc", name="tTc")
            nc.scalar.activation(tTc[:], cs, AF.Tanh)
            nc.vector.tensor_mul(hs, tSig[:, 1024:1536], tTc[:])
            if k == K - 1:
                nc.sync.dma_start(
                    out_d[j * 128:(j + 1) * 128, b * 512:(b + 1) * 512].bitcast(F32R),
                    hs,
                )

        # one uniform PSUM pool for the whole kernel: 2 slots x 4 banks
        psg = top.enter_context(tc.tile_pool(name="psg", bufs=2, space="PSUM"))

        # ---------------- prep: GLU -> u -> G table; step 0 interleaved ----------------
        with ExitStack() as prep:
            utp = prep.enter_context(tc.tile_pool(name="utp", bufs=1))
            wfp = prep.enter_context(tc.tile_pool(name="wfp", bufs=1))

            # x first on the DMA queue: transpose x tiles on PE straight after
            # DMA (no GLU in the critical path), then GLU on big transposed
            # tiles: uT = xaT * sigmoid(xbT)
            uT = [utp.tile([128, NT], F32R, tag=f"uT{ci}", name=f"uT{ci}")
                  for ci in range(2)]
            with ExitStack() as glu:
                xp = glu.enter_context(tc.tile_pool(name="xp2", bufs=2))
                xab = glu.enter_context(tc.tile_pool(name="xab", bufs=1))
                for half in range(2):
                    xa = [xab.tile([128, 512], F32, tag=f"xa{ci}", name=f"xa{ci}")
                          for ci in range(2)]
                    xb = [xab.tile([128, 512], F32, tag=f"xb{ci}", name=f"xb{ci}")
                          for ci in range(2)]
                    for tl in range(4):
                        ti = half * 4 + tl
                        xt = xp.tile([128, D], F32, tag="x", name="xt")
                        nc.sync.dma_start(xt[:], x_d[ti * 128:(ti + 1) * 128, :])
                        ptp = psg.tile([128, G4], F32, tag="P", name="Ptr")
                        for ci in range(4):
                            ptr = ptp[:, ci * 512:ci * 512 + 128]
                            nc.tensor.transpose(
                                ptr, xt[:, ci * 128:(ci + 1) * 128], ident_f32[:]
                            )
                            dst = xa[ci] if ci < 2 else xb[ci - 2]
                            nc.vector.tensor_copy(dst[:, tl * 128:(tl + 1) * 128], ptr)
                    for ci in range(2):
                        sgt = tp.tile([128, 512], F32, tag="tSig", name="sgt")
                        nc.scalar.activation(sgt[:], xb[ci][:], AF.Sigmoid)
                        nc.vector.tensor_mul(
                            uT[ci][:, half * 512:(half + 1) * 512], xa[ci][:], sgt[:]
                        )

            wf = []
            for ck in range(2):
                t = wfp.tile([128, G4], F32R, tag=f"wf{ck}", name=f"wf{ck}")
                nc.sync.dma_start(t[:], wf_d[ck * 128:(ck + 1) * 128, :].bitcast(F32R))
                wf.append(t)
            for dk in range(4):
                nc.sync.dma_start(
                    whh[dk][:], whh_d[dk * 128:(dk + 1) * 128, :].bitcast(F32R)
                )

            def g_phase(b):
                for i in range(16):
                    nc.scalar.activation(
                        gt[i][:, b * LW:b * LW + PAD], zeros8[:],
                        AF.Identity, bias=bias_sb[:, 16 + i:16 + i + 1],
                    )
                for j in range(4):
                    P = psg.tile([128, G4], F32, tag="P", name="Pg")
                    for q in range(4):
                        for ck in range(2):
                            nc.tensor.matmul(
                                P[:, q * 512:(q + 1) * 512],
                                wf[ck][:, (4 * q + j) * 128:(4 * q + j + 1) * 128],
                                uT[ck][:, b * 512:(b + 1) * 512],
                                start=(ck == 0), stop=(ck == 1),
                            )
                    for q in range(4):
                        nc.vector.tensor_scalar_add(
                            gt[4 * q + j][:, b * LW + PAD:b * LW + LW],
                            P[:, q * 512:(q + 1) * 512],
                            bias_sb[:, 4 * q + j:4 * q + j + 1],
                        )

            def unit(k, b, j):
                P = psg.tile([128, G4], F32, tag="P", name="P")
                # G slice first (ready early; starts each bank's group)
                for q in range(4):
                    nc.tensor.matmul(
                        P[:, q * 512:(q + 1) * 512], ident[:],
                        gt[4 * q + j][:, b * LW + k:b * LW + k + 512],
                        start=True, stop=False,
                    )
                for q in range(4):
                    for dk in range(4):
                        nc.tensor.matmul(
                            P[:, q * 512:(q + 1) * 512],
                            whh[dk][:, (4 * q + j) * 128:(4 * q + j + 1) * 128],
                            hT[(k + 1) % 2][dk][:, b * 512:(b + 1) * 512],
                            start=False, stop=(dk == 3),
                        )
                cell(j, b, P[:], k)

            # emission order keeps the PE fed while ACT/DVE run matmul-free
            # step-0 cells: G(b0), cell0(b0), G(b1), step1(b0), cell0(b1)
            g_phase(0)
            for j in range(4):
                cell0(j, 0)
            g_phase(1)
            for j in range(4):
                cell0(j, 1)

        # ---------------- LSTM steps 1..8 ----------------
        for k in range(1, K):
            for b in range(B_PER_CORE):
                for j in range(4):
                    unit(k, b, j)

    nc.compile()
    return nc


def _make_in_maps(inputs):
    x = np.asarray(inputs["x"], dtype=np.float32)
    conv_w = np.asarray(inputs["conv_w"], dtype=np.float64)
    conv_b = np.asarray(inputs["conv_b"], dtype=np.float64)
    w_ih = np.asarray(inputs["w_ih"], dtype=np.float64)
    w_hh = np.asarray(inputs["w_hh"], dtype=np.float32)
    b_ih = np.asarray(inputs["b_ih"], dtype=np.float64)
    b_hh = np.asarray(inputs["b_hh"], dtype=np.float64)

    # gate permutation: torch order i,f,g,o -> i,f,o,g
    perm = np.concatenate([
        np.arange(0, D), np.arange(D, 2 * D),
        np.arange(3 * D, 4 * D), np.arange(2 * D, 3 * D),
    ])
    wf = (w_ih @ conv_w)[perm]                                  # [2048, 256]
    bias_mm = (b_ih + b_hh + w_ih @ conv_b)[perm]               # real columns
    bias_pad = (b_ih + b_hh)[perm]                              # zero-padded columns
    whh_p = w_hh[perm]

    bias_both = np.concatenate([
        bias_mm.astype(np.float32).reshape(16, 128).T,
        bias_pad.astype(np.float32).reshape(16, 128).T,
    ], axis=1)                                                  # [128, 32]
    shared = {
        "wf": np.ascontiguousarray(wf.T.astype(np.float32)),            # [256, 2048]
        "whh": np.ascontiguousarray(whh_p.T.astype(np.float32)),        # [512, 2048]
        "bias": np.ascontiguousarray(bias_both),
    }
    in_maps = []
    for c in range(N_CORES):
        m = dict(shared)
        m["x"] = np.ascontiguousarray(
            x[c * B_PER_CORE:(c + 1) * B_PER_CORE].reshape(NT, D)
        )
        in_maps.append(m)
    return in_maps


def kernel(x, conv_w, conv_b, w_ih, w_hh, b_ih, b_hh):
    if "nc" not in _cache:
        _cache["nc"] = _build()
    nc = _cache["nc"]

    in_maps = _make_in_maps(dict(
        x=x, conv_w=conv_w, conv_b=conv_b, w_ih=w_ih, w_hh=w_hh,
        b_ih=b_ih, b_hh=b_hh,
    ))

    res = bass_utils.run_bass_kernel_spmd(nc, in_maps, core_ids=list(range(N_CORES)))
    out = np.concatenate(
        [np.ascontiguousarray(r["out"].T).reshape(B_PER_CORE, L, D)
         for r in res.results], axis=0
    )
    return out

